# revision 1
# baseline (speedup 1.0000x reference)
"""Trainium2 Bass kernel for nn_BatchAugmentation_53850299957920.

Pipeline (reference semantics):
  for a in 0..2: same-class/cross-dataset mixup -> dropout -> per-row scale ->
  per-gene scale -> +0.01*gaussian noise -> relu;  concat the 3 augs.

Split of work:
  * Host (exact jax-CPU threefry — the RNG must match the reference
    bit-exactly for every discrete decision): all random draws, the [B,B]
    mixup candidate search, per-row/per-gene scalars.  O(B*D) bytes of RNG
    are shipped compactly (noise + keep*gscale as f16).
  * Device (8 NeuronCores, data-parallel over the batch): all heavy
    elementwise math on [B, D] = [2048, 20000]:
        out[a,i,:] = relu(0.01*(gd[a,i,:]*(c1[a,i]*x[i,:] + c2[a,i]*xp[a,i,:])
                           + nz[a,i,:]))
    with c1 = 100*scale*lam_eff, c2 = 100*scale*(1-lam_eff),
    gd = keep*gscale (f16), nz = noise (f16), xp = parent rows of the mixup.
"""
import sys

if "/opt/trn_rl_repo" not in sys.path:
    sys.path.insert(0, "/opt/trn_rl_repo")

import numpy as np

N_AUG = 3
DROPOUT = 0.2
DS_MIN = 0.8
DS_MAX = 1.2
MIXUP_P = 0.3
ALPHA = 0.4
GENE_P = 0.2

N_CORES = 8
P = 128
B, D = 2048, 20000
R = B // N_CORES          # rows per core
T = R // P                # row-tiles per core
W = 2000                  # column chunk

_PROGRAM_CACHE = {}


# ----------------------------------------------------------------------------
# host side: exact RNG decomposition (jax CPU threefry == reference bits)
# ----------------------------------------------------------------------------
def _host_decompose(x, ctx, y):
    import jax
    import jax.numpy as jnp

    cpu = jax.devices("cpu")[0]
    Bx, Dx = x.shape
    base_key = jax.device_put(jax.random.key(42), cpu)

    c1 = np.empty((N_AUG, Bx), np.float32)
    c2 = np.empty((N_AUG, Bx), np.float32)
    j_sel_all = np.empty((N_AUG, Bx), np.int64)
    do_mix_all = np.empty((N_AUG, Bx), bool)
    ctxs = []
    keep_list = []
    gscale_list = []
    noise_list = []

    yy = np.asarray(y)
    cc = np.asarray(ctx)

    with jax.default_device(cpu):
        for a in range(N_AUG):
            k = jax.random.fold_in(base_key, a)
            k_mix, k_drop, k_scale, k_gmask, k_gscale, k_noise = \
                jax.random.split(k, 6)

            # --- mixup selection (must match reference bit-exactly) ---
            k_sel, k_pick, k_lam, k_coin = jax.random.split(k_mix, 4)
            mix_mask = np.asarray(jax.random.uniform(k_sel, (Bx,))) < MIXUP_P
            cand = (yy[None, :] == yy[:, None]) & (cc[None, :] != cc[:, None])
            counts = cand.sum(axis=1).astype(np.int32)
            u = np.asarray(jax.random.uniform(k_pick, (Bx,)))
            kth = np.minimum(
                np.floor(u * counts.astype(np.float32)).astype(np.int32),
                np.maximum(counts - 1, 0),
            )
            csum = np.cumsum(cand.astype(np.int32), axis=1)
            j_sel = np.argmax(csum > kth[:, None], axis=1)
            lam = np.asarray(
                jax.random.beta(k_lam, ALPHA, ALPHA, (Bx,), dtype=jnp.float32)
            )
            do_mix = mix_mask & (counts > 0)
            coin = np.asarray(jax.random.uniform(k_coin, (Bx,))) < 0.5
            ctx_new = np.where(do_mix & (~coin), cc[j_sel], cc).astype(cc.dtype)

            scale = (
                np.asarray(jax.random.uniform(k_scale, (Bx, 1)))[:, 0]
                * (DS_MAX - DS_MIN) + DS_MIN
            ).astype(np.float32)
            lam_eff = np.where(do_mix, lam, np.float32(1.0)).astype(np.float32)
            c1[a] = 100.0 * scale * lam_eff
            c2[a] = np.where(do_mix, 100.0 * scale * (1.0 - lam_eff), 0.0)
            j_sel_all[a] = j_sel
            do_mix_all[a] = do_mix
            ctxs.append(ctx_new)

            gmask = np.asarray(jax.random.uniform(k_gmask, (Dx,))) < GENE_P
            gs_draw = np.asarray(jax.random.uniform(k_gscale, (Dx,))) * 0.4 + 0.8
            gscale_list.append(np.where(gmask, gs_draw, 1.0).astype(np.float32))

            # big exact draws
            keep_list.append(
                np.asarray(jax.random.uniform(k_drop, (Bx, Dx))) > DROPOUT
            )
            noise_list.append(
                np.asarray(
                    jax.random.normal(k_noise, (Bx, Dx), dtype=jnp.float32)
                ).astype(np.float16)
            )

    return dict(c1=c1, c2=c2, j_sel=j_sel_all, do_mix=do_mix_all,
                ctxs=np.stack(ctxs, axis=0), keep=keep_list,
                gscale=gscale_list, noise=noise_list)


# ----------------------------------------------------------------------------
# device program
# ----------------------------------------------------------------------------
def _build_program(noise_engine="gpsimd"):
    import concourse.bacc as bacc
    from concourse import mybir
    from concourse.tile import TileContext

    A = N_AUG
    nc = bacc.Bacc("TRN2")
    f32, f16 = mybir.dt.float32, mybir.dt.float16
    t_xb = nc.dram_tensor("xb", [R, D], f32, kind="ExternalInput")
    t_xp = nc.dram_tensor("xp", [A, R, D], f32, kind="ExternalInput")
    t_gd = nc.dram_tensor("gd", [A, R, D], f16, kind="ExternalInput")
    t_nz = nc.dram_tensor("nz", [A, R, D], f16, kind="ExternalInput")
    t_c1 = nc.dram_tensor("c1", [A, T, P, 1], f32, kind="ExternalInput")
    t_c2 = nc.dram_tensor("c2", [A, T, P, 1], f32, kind="ExternalInput")
    t_out = nc.dram_tensor("out", [A, R, D], f32, kind="ExternalOutput")

    Copy = mybir.ActivationFunctionType.Copy
    Relu = mybir.ActivationFunctionType.Relu
    mult = mybir.AluOpType.mult
    add = mybir.AluOpType.add
    NCH = D // W

    with TileContext(nc) as tc:
        with (
            tc.tile_pool(name="coefs", bufs=1) as coef_pool,
            tc.tile_pool(name="io", bufs=3) as io_pool,
            tc.tile_pool(name="work", bufs=4) as work_pool,
        ):
            c1_sb, c2_sb = [], []
            for a in range(A):
                for t in range(T):
                    c1t = coef_pool.tile([P, 1], f32, name=f"c1_{a}_{t}")
                    nc.sync.dma_start(c1t[:, :], t_c1[a, t])
                    c2t = coef_pool.tile([P, 1], f32, name=f"c2_{a}_{t}")
                    nc.sync.dma_start(c2t[:, :], t_c2[a, t])
                    c1_sb.append(c1t)
                    c2_sb.append(c2t)

            for t in range(T):
                rows = slice(t * P, (t + 1) * P)
                for ch in range(NCH):
                    cols = slice(ch * W, (ch + 1) * W)
                    xb_c = io_pool.tile([P, W], f32, name="xb_c")
                    nc.sync.dma_start(xb_c[:, :], t_xb[rows, cols])
                    for a in range(A):
                        xp_c = io_pool.tile([P, W], f32, name="xp_c")
                        nc.sync.dma_start(xp_c[:, :], t_xp[a, rows, cols])
                        gd_c = io_pool.tile([P, W], f16, name="gd_c")
                        nc.sync.dma_start(gd_c[:, :], t_gd[a, rows, cols])
                        nz_c = io_pool.tile([P, W], f16, name="nz_c")
                        nc.sync.dma_start(nz_c[:, :], t_nz[a, rows, cols])

                        sx = work_pool.tile([P, W], f32, name="sx")
                        nc.scalar.activation(sx[:, :], xb_c[:, :], Copy,
                                             scale=c1_sb[a * T + t][:, :])
                        nc.vector.scalar_tensor_tensor(
                            sx[:, :], xp_c[:, :], c2_sb[a * T + t][:, :],
                            sx[:, :], mult, add)
                        nc.vector.tensor_tensor(sx[:, :], sx[:, :],
                                                gd_c[:, :], mult)
                        if noise_engine == "gpsimd":
                            nc.gpsimd.tensor_tensor(sx[:, :], sx[:, :],
                                                    nz_c[:, :], add)
                        else:
                            nc.vector.tensor_tensor(sx[:, :], sx[:, :],
                                                    nz_c[:, :], add)
                        out_c = io_pool.tile([P, W], f32, name="out_c")
                        nc.scalar.activation(out_c[:, :], sx[:, :], Relu,
                                             scale=0.01)
                        nc.sync.dma_start(t_out[a, rows, cols], out_c[:, :])
    nc.finalize()
    return nc


def _get_program():
    key = (R, D, W)
    if key not in _PROGRAM_CACHE:
        _PROGRAM_CACHE[key] = _build_program()
    return _PROGRAM_CACHE[key]


# ----------------------------------------------------------------------------
# entry point
# ----------------------------------------------------------------------------
def kernel(x, ctx, y, cont_covs, cat_covs):
    from concourse.bass_utils import run_bass_kernel_spmd

    x = np.ascontiguousarray(np.asarray(x, dtype=np.float32))
    ctx = np.asarray(ctx)
    y = np.asarray(y)
    assert x.shape == (B, D), x.shape

    dec = _host_decompose(x, ctx, y)

    # per-core inputs
    in_maps = []
    for c in range(N_CORES):
        rows = slice(c * R, (c + 1) * R)
        xp = np.zeros((N_AUG, R, D), np.float32)
        gd = np.empty((N_AUG, R, D), np.float16)
        nz = np.empty((N_AUG, R, D), np.float16)
        for a in range(N_AUG):
            m = dec["do_mix"][a, rows]
            xp[a][m] = x[dec["j_sel"][a, rows][m]]
            gd[a] = dec["keep"][a][rows] * dec["gscale"][a][None, :]
            nz[a] = dec["noise"][a][rows]
        in_maps.append({
            "xb": x[rows],
            "xp": xp,
            "gd": gd,
            "nz": nz,
            "c1": np.ascontiguousarray(
                dec["c1"][:, rows].reshape(N_AUG, T, P, 1)),
            "c2": np.ascontiguousarray(
                dec["c2"][:, rows].reshape(N_AUG, T, P, 1)),
        })

    nc = _get_program()
    res = run_bass_kernel_spmd(nc, in_maps, core_ids=list(range(N_CORES)))

    counts = np.empty((N_AUG * B, D), np.float32)
    for c in range(N_CORES):
        out_c = res.results[c]["out"]
        for a in range(N_AUG):
            counts[a * B + c * R:(a * B + (c + 1) * R)] = out_c[a]

    aug_ctxs = dec["ctxs"].reshape(-1)
    y_rep = np.tile(y, N_AUG)
    cont_rep = np.tile(np.asarray(cont_covs), (N_AUG, 1)).reshape(-1)
    cat_rep = np.tile(np.asarray(cat_covs), (N_AUG, 1)).reshape(-1)
    return counts, aug_ctxs, y_rep, cont_rep, cat_rep


# revision 2
# speedup vs baseline: 1.2994x; 1.2994x over previous
"""Trainium2 Bass kernel for nn_BatchAugmentation_53850299957920.

Reference pipeline (3 augs of x[2048, 20000]):
  same-class/cross-dataset mixup -> dropout -> per-row scale -> per-gene scale
  -> + 0.01*gaussian noise -> relu; concat augs (plus repeated ctx/y/cov
  metadata outputs).

Work split:
  * Host (exact jax-CPU threefry — RNG must match the reference bit-exactly
    for every discrete decision): all random draws, the [B,B] mixup candidate
    search, per-row/per-gene scalars.  The O(B*D) random tensors ship
    compactly: noise + keep*gscale as f16.
  * Device (8 NeuronCores, data-parallel over batch rows): all heavy [B,D]
    elementwise math.  The mixup row-blend runs on the (otherwise idle)
    tensor engine as selection matmuls so parent rows transfer compactly:

      psum = diag(c1) @ xb_tile + Sp @ xq_tile          (PE, per 512-col bank)
      out  = relu(0.01 * (gd * psum + nz))              (DVE / GPSIMD / ACT)

    with c1 = 100*scale*lam_eff, Sp[k,p] = 100*scale*(1-lam) for slot k ->
    row p, gd = keep*gscale (f16), nz = noise (f16).  xq holds up to 128
    deduplicated parent rows per 128-row tile (across all 3 augs); the rare
    overflow rows (cap exceeded) are patched on the host afterwards.
"""
import sys

if "/opt/trn_rl_repo" not in sys.path:
    sys.path.insert(0, "/opt/trn_rl_repo")

import numpy as np

N_AUG = 3
DROPOUT = 0.2
DS_MIN = 0.8
DS_MAX = 1.2
MIXUP_P = 0.3
ALPHA = 0.4
GENE_P = 0.2

N_CORES = 8
P = 128
B, D = 2048, 20000
R = B // N_CORES          # rows per core
T = R // P                # row-tiles per core
W = 2000                  # column chunk
BANK_F32 = 512            # one PSUM bank in f32 elements

_PROGRAM_CACHE = {}


# ----------------------------------------------------------------------------
# host side: exact RNG decomposition (jax CPU threefry == reference bits)
# ----------------------------------------------------------------------------
def _host_decompose(x, ctx, y):
    import jax
    import jax.numpy as jnp

    cpu = jax.devices("cpu")[0]
    Bx, Dx = x.shape
    base_key = jax.device_put(jax.random.key(42), cpu)

    c1 = np.empty((N_AUG, Bx), np.float32)
    c2 = np.empty((N_AUG, Bx), np.float32)
    j_sel_all = np.empty((N_AUG, Bx), np.int64)
    do_mix_all = np.empty((N_AUG, Bx), bool)
    ctxs = []
    keep_list = []
    gscale_list = []
    noise_list = []

    yy = np.asarray(y)
    cc = np.asarray(ctx)

    with jax.default_device(cpu):
        for a in range(N_AUG):
            k = jax.random.fold_in(base_key, a)
            k_mix, k_drop, k_scale, k_gmask, k_gscale, k_noise = \
                jax.random.split(k, 6)

            # --- mixup selection (bit-exact vs reference) ---
            k_sel, k_pick, k_lam, k_coin = jax.random.split(k_mix, 4)
            mix_mask = np.asarray(jax.random.uniform(k_sel, (Bx,))) < MIXUP_P
            cand = (yy[None, :] == yy[:, None]) & (cc[None, :] != cc[:, None])
            counts = cand.sum(axis=1).astype(np.int32)
            u = np.asarray(jax.random.uniform(k_pick, (Bx,)))
            kth = np.minimum(
                np.floor(u * counts.astype(np.float32)).astype(np.int32),
                np.maximum(counts - 1, 0),
            )
            csum = np.cumsum(cand.astype(np.int32), axis=1)
            j_sel = np.argmax(csum > kth[:, None], axis=1)
            lam = np.asarray(
                jax.random.beta(k_lam, ALPHA, ALPHA, (Bx,), dtype=jnp.float32)
            )
            do_mix = mix_mask & (counts > 0)
            coin = np.asarray(jax.random.uniform(k_coin, (Bx,))) < 0.5
            ctx_new = np.where(do_mix & (~coin), cc[j_sel], cc).astype(cc.dtype)

            scale = (
                np.asarray(jax.random.uniform(k_scale, (Bx, 1)))[:, 0]
                * (DS_MAX - DS_MIN) + DS_MIN
            ).astype(np.float32)
            lam_eff = np.where(do_mix, lam, np.float32(1.0)).astype(np.float32)
            c1[a] = 100.0 * scale * lam_eff
            c2[a] = np.where(do_mix, 100.0 * scale * (1.0 - lam_eff), 0.0)
            j_sel_all[a] = j_sel
            do_mix_all[a] = do_mix
            ctxs.append(ctx_new)

            gmask = np.asarray(jax.random.uniform(k_gmask, (Dx,))) < GENE_P
            gs_draw = np.asarray(jax.random.uniform(k_gscale, (Dx,))) * 0.4 + 0.8
            gscale_list.append(np.where(gmask, gs_draw, 1.0).astype(np.float32))

            # big exact draws
            keep_list.append(
                np.asarray(jax.random.uniform(k_drop, (Bx, Dx))) > DROPOUT
            )
            noise_list.append(
                np.asarray(
                    jax.random.normal(k_noise, (Bx, Dx), dtype=jnp.float32)
                ).astype(np.float16)
            )

    return dict(c1=c1, c2=c2, j_sel=j_sel_all, do_mix=do_mix_all,
                ctxs=np.stack(ctxs, axis=0), keep=keep_list,
                gscale=gscale_list, noise=noise_list)


# ----------------------------------------------------------------------------
# per-core input assembly
# ----------------------------------------------------------------------------
def _build_in_maps(x, dec):
    """Returns (in_maps, overflow) where overflow is a list of
    (a, global_row) whose parent didn't fit in the 128-slot cap."""
    in_maps = []
    overflow = []
    for c in range(N_CORES):
        rows = slice(c * R, (c + 1) * R)
        gd = np.empty((N_AUG, R, D), np.float16)
        nz = np.empty((N_AUG, R, D), np.float16)
        for a in range(N_AUG):
            gd[a] = dec["keep"][a][rows] * dec["gscale"][a][None, :]
            nz[a] = dec["noise"][a][rows]

        xq = np.zeros((T, P, D), np.float32)
        diag = np.zeros((N_AUG, T, P, P), np.float32)
        sp = np.zeros((N_AUG, T, P, P), np.float32)
        ar = np.arange(P)
        for t in range(T):
            slot_of = {}          # parent row j -> slot
            slot_js = []
            for a in range(N_AUG):
                base = c * R + t * P
                diag[a, t, ar, ar] = dec["c1"][a, base:base + P]
                mix_p = np.nonzero(dec["do_mix"][a, base:base + P])[0]
                for p in mix_p:
                    j = int(dec["j_sel"][a, base + p])
                    k = slot_of.get(j)
                    if k is None:
                        if len(slot_js) >= P:
                            overflow.append((a, base + p))
                            continue
                        k = len(slot_js)
                        slot_of[j] = k
                        slot_js.append(j)
                    sp[a, t, k, p] = dec["c2"][a, base + p]
            if slot_js:
                xq[t, :len(slot_js)] = x[np.asarray(slot_js)]

        in_maps.append({
            "xb": x[rows], "xq": xq, "gd": gd, "nz": nz,
            "diag": diag, "sp": sp,
        })
    return in_maps, overflow


def _patch_overflow(counts, x, dec, overflow):
    """Host-exact recompute of rows whose mixup parent had no xq slot."""
    for a, i in overflow:
        gd_row = (dec["keep"][a][i] * dec["gscale"][a]).astype(np.float16)
        s = dec["c1"][a, i] * x[i] + dec["c2"][a, i] * x[dec["j_sel"][a, i]]
        h = gd_row.astype(np.float32) * s + \
            dec["noise"][a][i].astype(np.float32)
        counts[a * B + i] = np.maximum(np.float32(0.01) * h, 0.0)


# ----------------------------------------------------------------------------
# device program (v2: PE selection-matmul mixup)
# ----------------------------------------------------------------------------
def _build_program(noise_engine="gpsimd"):
    import concourse.bacc as bacc
    from concourse import mybir
    from concourse.tile import TileContext

    A = N_AUG
    nc = bacc.Bacc("TRN2")
    f32, f16 = mybir.dt.float32, mybir.dt.float16
    t_xb = nc.dram_tensor("xb", [R, D], f32, kind="ExternalInput")
    t_xq = nc.dram_tensor("xq", [T, P, D], f32, kind="ExternalInput")
    t_gd = nc.dram_tensor("gd", [A, R, D], f16, kind="ExternalInput")
    t_nz = nc.dram_tensor("nz", [A, R, D], f16, kind="ExternalInput")
    t_diag = nc.dram_tensor("diag", [A, T, P, P], f32, kind="ExternalInput")
    t_sp = nc.dram_tensor("sp", [A, T, P, P], f32, kind="ExternalInput")
    t_out = nc.dram_tensor("out", [A, R, D], f32, kind="ExternalOutput")

    Relu = mybir.ActivationFunctionType.Relu
    mult = mybir.AluOpType.mult
    add = mybir.AluOpType.add
    NCH = D // W
    NSUB = (W + BANK_F32 - 1) // BANK_F32
    WPAD = NSUB * BANK_F32

    with TileContext(nc) as tc:
        with (
            tc.tile_pool(name="wts", bufs=1) as wt_pool,
            tc.tile_pool(name="io", bufs=3) as io_pool,
            tc.tile_pool(name="io16", bufs=4) as io16_pool,
            tc.tile_pool(name="work", bufs=4) as work_pool,
            tc.tile_pool(name="psum", bufs=2, space="PSUM") as psum_pool,
        ):
            diag_sb, sp_sb = [], []
            for a in range(A):
                for t in range(T):
                    dg = wt_pool.tile([P, P], f32, name=f"diag_{a}_{t}")
                    nc.sync.dma_start(dg[:, :], t_diag[a, t])
                    spt = wt_pool.tile([P, P], f32, name=f"sp_{a}_{t}")
                    nc.sync.dma_start(spt[:, :], t_sp[a, t])
                    diag_sb.append(dg)
                    sp_sb.append(spt)

            for t in range(T):
                rows = slice(t * P, (t + 1) * P)
                for ch in range(NCH):
                    cols = slice(ch * W, (ch + 1) * W)
                    xb_c = io_pool.tile([P, W], f32, name="xb_c")
                    nc.sync.dma_start(xb_c[:, :], t_xb[rows, cols])
                    xq_c = io_pool.tile([P, W], f32, name="xq_c")
                    nc.sync.dma_start(xq_c[:, :], t_xq[t, :, cols])
                    for a in range(A):
                        gd_c = io16_pool.tile([P, W], f16, name="gd_c")
                        nc.sync.dma_start(gd_c[:, :], t_gd[a, rows, cols])
                        nz_c = io16_pool.tile([P, W], f16, name="nz_c")
                        nc.sync.dma_start(nz_c[:, :], t_nz[a, rows, cols])

                        ps = psum_pool.tile([P, WPAD], f32, name="ps")
                        for s in range(NSUB):
                            lo = s * BANK_F32
                            hi = min(W, lo + BANK_F32)
                            nc.tensor.matmul(
                                ps[:, lo:hi], diag_sb[a * T + t][:, :],
                                xb_c[:, lo:hi], start=True, stop=False)
                        for s in range(NSUB):
                            lo = s * BANK_F32
                            hi = min(W, lo + BANK_F32)
                            nc.tensor.matmul(
                                ps[:, lo:hi], sp_sb[a * T + t][:, :],
                                xq_c[:, lo:hi], start=False, stop=True)

                        sx = work_pool.tile([P, W], f32, name="sx")
                        nc.vector.tensor_tensor(sx[:, :], ps[:, 0:W],
                                                gd_c[:, :], mult)
                        if noise_engine == "gpsimd":
                            nc.gpsimd.tensor_tensor(sx[:, :], sx[:, :],
                                                    nz_c[:, :], add)
                        else:
                            nc.vector.tensor_tensor(sx[:, :], sx[:, :],
                                                    nz_c[:, :], add)
                        out_c = io_pool.tile([P, W], f32, name="out_c")
                        nc.scalar.activation(out_c[:, :], sx[:, :], Relu,
                                             scale=0.01)
                        nc.sync.dma_start(t_out[a, rows, cols], out_c[:, :])
    nc.finalize()
    return nc


def _get_program():
    key = (R, D, W, "v2")
    if key not in _PROGRAM_CACHE:
        _PROGRAM_CACHE[key] = _build_program()
    return _PROGRAM_CACHE[key]


# ----------------------------------------------------------------------------
# entry point
# ----------------------------------------------------------------------------
def kernel(x, ctx, y, cont_covs, cat_covs):
    from concourse.bass_utils import run_bass_kernel_spmd

    x = np.ascontiguousarray(np.asarray(x, dtype=np.float32))
    ctx = np.asarray(ctx)
    y = np.asarray(y)
    assert x.shape == (B, D), x.shape

    dec = _host_decompose(x, ctx, y)
    in_maps, overflow = _build_in_maps(x, dec)

    nc = _get_program()
    res = run_bass_kernel_spmd(nc, in_maps, core_ids=list(range(N_CORES)))

    counts = np.empty((N_AUG * B, D), np.float32)
    for c in range(N_CORES):
        out_c = res.results[c]["out"]
        for a in range(N_AUG):
            counts[a * B + c * R:(a * B + (c + 1) * R)] = out_c[a]
    _patch_overflow(counts, x, dec, overflow)

    aug_ctxs = dec["ctxs"].reshape(-1)
    y_rep = np.tile(y, N_AUG)
    cont_rep = np.tile(np.asarray(cont_covs), (N_AUG, 1)).reshape(-1)
    cat_rep = np.tile(np.asarray(cat_covs), (N_AUG, 1)).reshape(-1)
    return counts, aug_ctxs, y_rep, cont_rep, cat_rep
